# revision 32
# baseline (speedup 1.0000x reference)
"""Multi-head attention (N=2, L=2048, 16 heads x 64) on 8 TRN2 NeuronCores.

Sharding: batch x head-group. Core c handles batch c//4 and heads
4*(c%4)..4*(c%4)+3, so each core streams only its own batch's x/y (8.4 MB
instead of 16.8 MB) and computes all four projections for its head slice.
One AllToAll per 4-core batch group (replica_groups [[0-3],[4-7]]) reshards
from head-split to sequence-split for the output projection.

All matmuls bf16 with fp32 accumulation; softmax in fp32 (exp on ScalarE,
denominator via a ones-column appended to V). Scores are computed
transposed ([k, q]); attention weights feed the AV matmul as the moving
operand. The AV output [dv+1, q] carries the denominator in row 64;
normalization is 1/den broadcast via DMA and applied on VectorE. The
output projection produces the final output transposed; the host
transposes back.
"""
import sys

sys.path.insert(0, "/opt/trn_rl_repo")

import numpy as np
import ml_dtypes

import concourse.bass as bass
import concourse.bacc as bacc
import concourse.mybir as mybir
import concourse.tile as tile
from concourse.bass_utils import run_bass_kernel_spmd

BF16 = ml_dtypes.bfloat16

DM = 1024      # dmodel
DK = 64        # head dim
H = 16         # heads
NB = 2         # batch
L = 2048       # seq len
NC = 8         # cores
GPB = NC // NB  # head-groups (cores) per batch = 4
HPC = H // GPB  # heads per core = 4
DPC = HPC * DK  # depth per core = 256
VW = HPC * 65   # v_aug width (4 heads x (64+1 ones col)) = 260

SW = 512       # sub-window (AV, masks, output chunk)
WW = 1024      # score/exp window
KT = 128       # k tile
NSW = L // SW   # 4 sub-windows
NWW = L // WW   # 2 score windows
NKT = L // KT   # 16 k tiles
CHUNK = L // GPB  # 512 q rows per core output

_CACHE = {}


def _classify_blocks(mask):
    """Per 512-granular (qs, kt): 0=skip, 1=full, 2=partial (+ q-span, pattern)."""
    mask = np.asarray(mask, dtype=bool)
    cls = [[0] * NKT for _ in range(NSW)]
    span = [[None] * NKT for _ in range(NSW)]
    pat_ids = {}
    pats = []
    pat_idx = [[-1] * NKT for _ in range(NSW)]
    for qs in range(NSW):
        for kt in range(NKT):
            sub = mask[qs * SW:(qs + 1) * SW, kt * KT:(kt + 1) * KT]
            rows = np.nonzero(sub.any(axis=1))[0]
            if rows.size == 0:
                cls[qs][kt] = 0
            elif sub.all():
                cls[qs][kt] = 1
                span[qs][kt] = (0, SW)
            else:
                cls[qs][kt] = 2
                span[qs][kt] = (int(rows[0]), int(rows[-1]) + 1)
                pat = np.ascontiguousarray(sub.T).astype(BF16)  # [128 k, SW q]
                key = pat.tobytes()
                if key not in pat_ids:
                    pat_ids[key] = len(pats)
                    pats.append(pat)
                pat_idx[qs][kt] = pat_ids[key]
    # general-mask safety: the first included kt of each sub-window must cover
    # the full 512 columns (its start=True matmul clears PSUM has_written)
    for qs in range(NSW):
        for kt in range(NKT):
            if cls[qs][kt]:
                span[qs][kt] = (0, SW)
                break
    if not pats:
        pats.append(np.ones((KT, SW), dtype=BF16))
    return cls, span, pat_idx, np.stack(pats)


def _build(cls, span, pat_idx, n_pat):
    nc = bacc.Bacc("TRN2", target_bir_lowering=False, debug=False,
                   enable_asserts=False, num_devices=NC)
    f32, bf16 = mybir.dt.float32, mybir.dt.bfloat16

    # weights arrive host-pretransposed as [128, 8*cols] so each loads in a
    # single contiguous DMA
    xtb = nc.dram_tensor("xtb", [DM, L], bf16, kind="ExternalInput")
    ytb = nc.dram_tensor("ytb", [DM, L], bf16, kind="ExternalInput")
    wq = nc.dram_tensor("wq", [128, 8 * DPC], bf16, kind="ExternalInput")
    wk = nc.dram_tensor("wk", [128, 8 * DPC], bf16, kind="ExternalInput")
    wv = nc.dram_tensor("wv", [128, 8 * VW], bf16, kind="ExternalInput")
    wo = nc.dram_tensor("wo", [128, 8 * DM], bf16, kind="ExternalInput")
    bqd = nc.dram_tensor("bq", [128, 2], f32, kind="ExternalInput")
    bkd = nc.dram_tensor("bk", [128, 2], f32, kind="ExternalInput")
    bv1 = nc.dram_tensor("bv1", [1, VW], bf16, kind="ExternalInput")
    bod = nc.dram_tensor("bo", [128, 8], f32, kind="ExternalInput")
    mpat = nc.dram_tensor("mpat", [KT, n_pat * SW], bf16, kind="ExternalInput")
    out_t = nc.dram_tensor("out_t", [DM, CHUNK], f32, kind="ExternalOutput")

    # per score-window (qw): which kt are included, and the union span of
    # valid q columns in window coordinates
    def window_kts(qw):
        out = []
        for kt in range(NKT):
            lo, hi = None, None
            for s in range(WW // SW):
                qs = qw * (WW // SW) + s
                if cls[qs][kt]:
                    a, b = span[qs][kt]
                    a += s * SW
                    b += s * SW
                    lo = a if lo is None else min(lo, a)
                    hi = b if hi is None else max(hi, b)
            if lo is not None:
                out.append((kt, lo, hi))
        return out

    with tile.TileContext(nc) as tc:
        with (
            tc.tile_pool(name="const", bufs=1) as cst,
            tc.tile_pool(name="xy", bufs=24) as xy,
            tc.tile_pool(name="big", bufs=1) as big,
            tc.tile_pool(name="exp", bufs=14) as expp,
            tc.tile_pool(name="sm", bufs=4) as sm,
            tc.tile_pool(name="osb", bufs=3) as osb,
            tc.tile_pool(name="rhsp", bufs=8) as rhsp,
            tc.tile_pool(name="sp", bufs=2, space="PSUM") as sp,
            tc.tile_pool(name="avp", bufs=4, space="PSUM") as avp,
            tc.tile_pool(name="dram", bufs=1, space="DRAM") as dram,
            tc.tile_pool(name="dscr", bufs=4, space="DRAM") as dscrp,
        ):
            # ---- wq first: the very first projection needs it ----
            wq_sb = cst.tile([128, 8 * DPC], bf16)
            nc.sync.dma_start(wq_sb[:], wq[:])

            wo_sb = cst.tile([128, 8 * DM], bf16)
            bo_sb = cst.tile([128, 8], f32)

            # start-of-kernel barrier: absorbs per-core launch skew while the
            # big input DMAs stream, so the real AllToAll later isn't skewed
            bar_in = dram.tile([1, 8], f32)
            bar_out = dram.tile([1, 8], f32)
            barrier_sb = cst.tile([1, 8], f32, tag="barrier_sb")
            nc.vector.memset(barrier_sb[:], 0.0)
            nc.sync.dma_start(bar_in[:], barrier_sb[:])
            nc.gpsimd.collective_compute(
                "AllReduce", mybir.AluOpType.add,
                replica_groups=[list(range(NC))],
                ins=[bar_in.opt()], outs=[bar_out.opt()])

            # head-pair p = heads {2p, 2p+1} of this core's slice; each pair
            # shares one 128-row tile (rows 0-63 / 64-127)
            qT = [big.tile([128, L], bf16, name=f"qT{p}") for p in range(2)]
            kT = [big.tile([128, L], bf16, name=f"kT{p}") for p in range(2)]
            vaug = big.tile([128, NKT * VW], bf16)
            headT = [big.tile([128, L], bf16, name=f"hT{p}") for p in range(2)]

            # ---- projections (qc-outer so x and y stream interleaved and
            # all 16 HW DMA queues fill from t=0) ----
            for qc in range(L // 1024):
                xtiles, ytiles = [], []
                for dt in range(8):
                    t = xy.tile([128, 1024], bf16, tag="xy", name=f"x{qc}_{dt}")
                    nc.sync.dma_start(t[:], xtb[dt * 128:(dt + 1) * 128,
                                              qc * 1024:(qc + 1) * 1024])
                    xtiles.append(t)
                for dt in range(8):
                    t = xy.tile([128, 1024], bf16, tag="xy", name=f"y{qc}_{dt}")
                    nc.sync.dma_start(t[:], ytb[dt * 128:(dt + 1) * 128,
                                              qc * 1024:(qc + 1) * 1024])
                    ytiles.append(t)
                if qc == 0:
                    # remaining small constants, queued behind the first input
                    # wave (each a single contiguous DMA)
                    wk_sb = cst.tile([128, 8 * DPC], bf16)
                    wv_sb = cst.tile([128, 8 * VW], bf16)
                    nc.sync.dma_start(wk_sb[:], wk[:])
                    nc.sync.dma_start(wv_sb[:], wv[:])
                    bq_sb = cst.tile([128, 2], f32)
                    bk_sb = cst.tile([128, 2], f32)
                    nc.sync.dma_start(bq_sb[:], bqd[:])
                    nc.sync.dma_start(bk_sb[:], bkd[:])
                    bv1_sb = cst.tile([1, VW], bf16)
                    nc.sync.dma_start(bv1_sb[:], bv1[:])
                    mpat_sb = cst.tile([KT, n_pat * SW], bf16)
                    nc.sync.dma_start(mpat_sb[:], mpat[:])
                    ones_row = cst.tile([1, 128], bf16)
                    nc.vector.memset(ones_row[:], 1.0)
                for s in range(2):
                    col = qc * 1024 + s * SW
                    for src, wsb, bias, dst in ((xtiles, wq_sb, bq_sb, qT),
                                                (ytiles, wk_sb, bk_sb, kT)):
                        for p in range(2):
                            ps = sp.tile([128, WW], f32, tag="sp")
                            for dt in range(8):
                                nc.tensor.matmul(
                                    ps[:128, :SW],
                                    wsb[:, dt * DPC + p * 128:dt * DPC + (p + 1) * 128],
                                    src[dt][:, s * SW:(s + 1) * SW],
                                    start=(dt == 0), stop=(dt == 7))
                            nc.scalar.activation(dst[p][:, col:col + SW], ps[:128, :SW],
                                                 mybir.ActivationFunctionType.Identity,
                                                 bias=bias[:, p:p + 1])
                for j in range(8):
                    kti = qc * 8 + j
                    psv = avp.tile([128, VW], f32, tag="avp")
                    for dt in range(8):
                        nc.tensor.matmul(psv[:, :VW],
                                         ytiles[dt][:, j * KT:(j + 1) * KT],
                                         wv_sb[:, dt * VW:(dt + 1) * VW],
                                         start=(dt == 0), stop=False)
                    nc.tensor.matmul(psv[:, :VW], ones_row[:],
                                     bv1_sb[:], start=False, stop=True)
                    nc.vector.tensor_copy(vaug[:, kti * VW:kti * VW + VW],
                                          psv[:, :VW])

            # wo/bo issued after all input-stream DMAs (needed only at the end)
            nc.sync.dma_start(wo_sb[:], wo[:])
            nc.sync.dma_start(bo_sb[:], bod[:])

            # ---- attention (this core's batch only) ----
            # Two half A2As, one per score window. Shard j of A2A h = this
            # core's heads for q in [1024h + 128j, 1024h + 128j + 128) of its
            # batch. Core j then holds full depth of q-range 1024h + 128j for
            # BOTH batches (shards 0-3 carry batch 0, 4-7 carry batch 1), so
            # A2A#1 and the first output-projection half overlap window 1.
            QQ = 128
            a2aA_in = dram.tile([NC, DPC, QQ], bf16)
            a2aA_out = dram.tile([NC, DPC, QQ], bf16)
            a2aB_in = dram.tile([NC, DPC, QQ], bf16)
            a2aB_out = dram.tile([NC, DPC, QQ], bf16)

            def emit_oproj(half, aout):
                rhs = []
                for r in range(8):
                    jj, sub = r // 2, r % 2
                    t = rhsp.tile([128, 2 * QQ], bf16, tag="a2a",
                                  name=f"a2a{half}_{r}")
                    nc.sync.dma_start(t[:, 0:QQ],
                                      aout[jj][sub * 128:(sub + 1) * 128, :])
                    nc.sync.dma_start(t[:, QQ:2 * QQ],
                                      aout[jj + 4][sub * 128:(sub + 1) * 128, :])
                    rhs.append(t)
                for mt in range(8):
                    ps = sp.tile([128, WW], f32, tag="sp", name=f"po{half}_{mt}")
                    for r in range(8):
                        nc.tensor.matmul(
                            ps[:, :2 * QQ],
                            wo_sb[:, r * DM + mt * 128:r * DM + (mt + 1) * 128],
                            rhs[r][:],
                            start=(r == 0), stop=(r == 7))
                    ob = osb.tile([128, 2 * QQ], f32, tag="osb")
                    nc.vector.tensor_scalar_add(ob[:], ps[:, :2 * QQ],
                                                bo_sb[:, mt:mt + 1])
                    nc.sync.dma_start(
                        out_t[mt * 128:(mt + 1) * 128,
                              half * 2 * QQ:(half + 1) * 2 * QQ], ob[:])
            for qw in range(NWW):
                    wkts = window_kts(qw)
                    wcol = qw * WW
                    # last kt per sub-window (closes that AV chain)
                    lastkt = {}
                    for s in range(WW // SW):
                        qs = qw * (WW // SW) + s
                        inc = [kt for kt, _, _ in wkts if cls[qs][kt]]
                        if inc:
                            lastkt[s] = inc[-1]
                    # one pass per head-pair: scores -> exp -> AV per kt, so
                    # exp tiles die immediately and AV chains (2 qs halves x
                    # 2 heads = 4 PSUM banks) stay open across the kt loop
                    for p in range(2):
                        av_t = {}
                        for kt, lo, hi in wkts:
                            pss = []
                            for hp in range(2):
                                hs = hp * DK
                                ps = sp.tile([128, WW], f32, tag="sp", name=f"ps{hp}")
                                for a, b in ((lo, min(hi, SW)), (max(lo, SW), hi)):
                                    if a >= b:
                                        continue
                                    nc.tensor.matmul(
                                        ps[:KT, a:b],
                                        kT[p][hs:hs + DK, kt * KT:(kt + 1) * KT],
                                        qT[p][hs:hs + DK, wcol + a:wcol + b],
                                        start=True, stop=True)
                                pss.append(ps)
                            ets = []
                            for hp in range(2):
                                et = expp.tile([KT, WW], bf16, tag="exp")
                                nc.scalar.activation(et[:, lo:hi], pss[hp][:KT, lo:hi],
                                                     mybir.ActivationFunctionType.Exp)
                                ets.append(et)
                            for s in range(WW // SW):
                                qs = qw * (WW // SW) + s
                                if not cls[qs][kt]:
                                    continue
                                a, b = span[qs][kt]
                                for hp in range(2):
                                    h = 2 * p + hp  # head within this core
                                    et = ets[hp]
                                    if cls[qs][kt] == 2:
                                        pi = pat_idx[qs][kt]
                                        nc.vector.tensor_tensor(
                                            et[:, s * SW + a:s * SW + b],
                                            et[:, s * SW + a:s * SW + b],
                                            mpat_sb[:, pi * SW + a:pi * SW + b],
                                            mybir.AluOpType.mult)
                                    first = (s, hp) not in av_t
                                    if first:
                                        av_t[s, hp] = avp.tile(
                                            [65, SW], f32, tag="avp",
                                            name=f"av{s}{hp}")
                                    av = av_t[s, hp]
                                    done = kt == lastkt[s]
                                    nc.tensor.matmul(
                                        av[:, a:b],
                                        vaug[:, kt * VW + h * 65:kt * VW + (h + 1) * 65],
                                        et[:, s * SW + a:s * SW + b],
                                        start=first, stop=done)
                                    if not done:
                                        continue
                                    # chain closed: stage to SBUF immediately
                                    # (frees the PSUM bank), then normalize:
                                    # reshape den row across lanes, exact
                                    # reciprocal, broadcast via stride-0 DMA.
                                    stg = sm.tile([65, SW], f32, tag="stg")
                                    nc.vector.tensor_copy(stg[:], av[:, :])
                                    d128 = sm.tile([128, SW // 128], f32, tag="d128")
                                    nc.sync.dma_start(d128[:], stg[64:65, :])
                                    r128 = sm.tile([128, SW // 128], f32, tag="r128")
                                    nc.vector.reciprocal(r128[:], d128[:])
                                    dsc = dscrp.tile([1, SW], f32, tag="dscr")
                                    nc.sync.dma_start(dsc[:], r128[:])
                                    bcs = sm.tile([DK, SW], f32, tag="bcs")
                                    nc.sync.dma_start(bcs[:], dsc[:].to_broadcast([DK, SW]))
                                    ccol = qs * SW
                                    nc.vector.tensor_tensor(
                                        headT[p][hp * DK:(hp + 1) * DK, ccol:ccol + SW],
                                        stg[:DK, :], bcs[:],
                                        mybir.AluOpType.mult)
                    # all 4 heads done for this window: ship its 8 shards and
                    # fire this window's half A2A; the first half's output
                    # projection overlaps window 1's attention
                    ain = a2aA_in if qw == 0 else a2aB_in
                    for j in range(8):
                        for p in range(2):
                            nc.sync.dma_start(
                                ain[j][p * 128:(p + 1) * 128, :],
                                headT[p][:, qw * WW + j * QQ:qw * WW + (j + 1) * QQ])
                    if qw == 0:
                        nc.gpsimd.collective_compute(
                            "AllToAll", mybir.AluOpType.bypass,
                            replica_groups=[list(range(NC))],
                            ins=[a2aA_in.opt()], outs=[a2aA_out.opt()])
                        emit_oproj(0, a2aA_out)
                    else:
                        nc.gpsimd.collective_compute(
                            "AllToAll", mybir.AluOpType.bypass,
                            replica_groups=[list(range(NC))],
                            ins=[a2aB_in.opt()], outs=[a2aB_out.opt()])

            # keep the PE clock warm during the final collective wait (gated
            # on the LAST-written headT columns = end of attention)
            for i in range(14):
                wps = sp.tile([128, WW], f32, tag="sp", name=f"warm{i}")
                nc.tensor.matmul(wps[:, :SW], headT[1][:, L - 128:L],
                                 headT[1][:, L - SW:L],
                                 start=True, stop=True)

            emit_oproj(1, a2aB_out)

    nc.compile()
    return nc


def kernel(x, y, mask, Wq, bq, Wk, bk, Wv, bv, Wo, bo, _trace=False):
    x = np.asarray(x, np.float32)
    y = np.asarray(y, np.float32)
    cls, span, pat_idx, pats = _classify_blocks(mask)

    key = (x.shape,
           tuple(tuple(c) for c in cls),
           tuple(tuple(s) for s in span),
           tuple(tuple(p) for p in pat_idx),
           pats.tobytes())
    if key not in _CACHE:
        _CACHE[key] = _build(cls, span, pat_idx, pats.shape[0])
    nc = _CACHE[key]

    fac = np.float32(1.0 / np.sqrt(DK))
    xtb = [np.ascontiguousarray(x[n].T).astype(BF16) for n in range(NB)]
    ytb = [np.ascontiguousarray(y[n].T).astype(BF16) for n in range(NB)]
    Wq32 = np.asarray(Wq, np.float32) * fac
    bq32 = np.asarray(bq, np.float32) * fac

    def fold(w):
        # [DM, C] -> [128, 8*C]: dm-block dt lands at columns [dt*C, (dt+1)*C)
        c = w.shape[1]
        return np.ascontiguousarray(
            w.reshape(8, 128, c).transpose(1, 0, 2).reshape(128, 8 * c))

    mpat_f = np.ascontiguousarray(
        pats.transpose(1, 0, 2).reshape(KT, -1))  # [128, n_pat*SW]

    in_maps = []
    for c in range(NC):
        b, g = c // GPB, c % GPB
        d0 = g * DPC  # this core's depth slice (heads 4g..4g+3)
        wv_aug = np.zeros((DM, VW), np.float32)
        bv1 = np.zeros((1, VW), np.float32)
        for hp in range(HPC):
            h = HPC * g + hp
            wv_aug[:, hp * 65:hp * 65 + DK] = np.asarray(Wv, np.float32)[:, h * DK:(h + 1) * DK]
            bv1[0, hp * 65:hp * 65 + DK] = np.asarray(bv, np.float32)[h * DK:(h + 1) * DK]
            bv1[0, hp * 65 + DK] = 1.0
        in_maps.append({
            "xtb": xtb[b], "ytb": ytb[b],
            "wq": fold(Wq32[:, d0:d0 + DPC].astype(BF16)),
            "wk": fold(np.asarray(Wk, np.float32)[:, d0:d0 + DPC].astype(BF16)),
            "wv": fold(wv_aug.astype(BF16)),
            "wo": fold(np.asarray(Wo, np.float32).astype(BF16)),
            "bq": np.ascontiguousarray(bq32[d0:d0 + DPC].reshape(2, 128).T),
            "bk": np.ascontiguousarray(
                np.asarray(bk, np.float32)[d0:d0 + DPC].reshape(2, 128).T),
            "bv1": bv1.astype(BF16),
            "bo": np.ascontiguousarray(
                np.asarray(bo, np.float32).reshape(8, 128).T),
            "mpat": mpat_f,
        })

    res = run_bass_kernel_spmd(nc, in_maps, core_ids=list(range(NC)), trace=_trace)
    out = np.empty((NB, L, DM), np.float32)
    QQ = 128  # q rows per core per window half
    for c in range(NC):
        ot = res.results[c]["out_t"]
        for half in range(2):
            q0 = half * (L // 2) + QQ * c
            out[0, q0:q0 + QQ, :] = ot[:, half * 2 * QQ:half * 2 * QQ + QQ].T
            out[1, q0:q0 + QQ, :] = ot[:, half * 2 * QQ + QQ:half * 2 * QQ + 2 * QQ].T
    if _trace:
        kernel.last_results = res
    return out


# revision 35
# speedup vs baseline: 1.2311x; 1.2311x over previous
"""Multi-head attention (N=2, L=2048, 16 heads x 64) on 8 TRN2 NeuronCores.

Sharding: batch x head-group. Core c handles batch c//4 and heads
4*(c%4)..4*(c%4)+3, so each core streams only its own batch's x/y (8.4 MB
instead of 16.8 MB) and computes all four projections for its head slice.
One AllToAll per 4-core batch group (replica_groups [[0-3],[4-7]]) reshards
from head-split to sequence-split for the output projection.

All matmuls bf16 with fp32 accumulation; softmax in fp32 (exp on ScalarE,
denominator via a ones-column appended to V). Scores are computed
transposed ([k, q]); attention weights feed the AV matmul as the moving
operand. The AV output [dv+1, q] carries the denominator in row 64;
normalization is 1/den broadcast via DMA and applied on VectorE. The
output projection produces the final output transposed; the host
transposes back.
"""
import sys

sys.path.insert(0, "/opt/trn_rl_repo")

import numpy as np
import ml_dtypes

import concourse.bass as bass
import concourse.bacc as bacc
import concourse.mybir as mybir
import concourse.tile as tile
from concourse.bass_utils import run_bass_kernel_spmd

BF16 = ml_dtypes.bfloat16

DM = 1024      # dmodel
DK = 64        # head dim
H = 16         # heads
NB = 2         # batch
L = 2048       # seq len
NC = 8         # cores
GPB = NC // NB  # head-groups (cores) per batch = 4
HPC = H // GPB  # heads per core = 4
DPC = HPC * DK  # depth per core = 256
VW = HPC * 65   # v_aug width (4 heads x (64+1 ones col)) = 260

SW = 512       # sub-window (AV, masks, output chunk)
WW = 1024      # score/exp window
KT = 128       # k tile
NSW = L // SW   # 4 sub-windows
NWW = L // WW   # 2 score windows
NKT = L // KT   # 16 k tiles
CHUNK = L // GPB  # 512 q rows per core output

_CACHE = {}


def _classify_blocks(mask):
    """Per 512-granular (qs, kt): 0=skip, 1=full, 2=partial (+ q-span, pattern)."""
    mask = np.asarray(mask, dtype=bool)
    cls = [[0] * NKT for _ in range(NSW)]
    span = [[None] * NKT for _ in range(NSW)]
    pat_ids = {}
    pats = []
    pat_idx = [[-1] * NKT for _ in range(NSW)]
    for qs in range(NSW):
        for kt in range(NKT):
            sub = mask[qs * SW:(qs + 1) * SW, kt * KT:(kt + 1) * KT]
            rows = np.nonzero(sub.any(axis=1))[0]
            if rows.size == 0:
                cls[qs][kt] = 0
            elif sub.all():
                cls[qs][kt] = 1
                span[qs][kt] = (0, SW)
            else:
                cls[qs][kt] = 2
                span[qs][kt] = (int(rows[0]), int(rows[-1]) + 1)
                pat = np.ascontiguousarray(sub.T).astype(BF16)  # [128 k, SW q]
                key = pat.tobytes()
                if key not in pat_ids:
                    pat_ids[key] = len(pats)
                    pats.append(pat)
                pat_idx[qs][kt] = pat_ids[key]
    # general-mask safety: the first included kt of each sub-window must cover
    # the full 512 columns (its start=True matmul clears PSUM has_written)
    for qs in range(NSW):
        for kt in range(NKT):
            if cls[qs][kt]:
                span[qs][kt] = (0, SW)
                break
    if not pats:
        pats.append(np.ones((KT, SW), dtype=BF16))
    return cls, span, pat_idx, np.stack(pats)


def _build(cls, span, pat_idx, n_pat):
    nc = bacc.Bacc("TRN2", target_bir_lowering=False, debug=False,
                   enable_asserts=False, num_devices=NC)
    f32, bf16 = mybir.dt.float32, mybir.dt.bfloat16

    # weights arrive host-pretransposed as [128, 8*cols] so each loads in a
    # single contiguous DMA
    xtb = nc.dram_tensor("xtb", [DM, L], bf16, kind="ExternalInput")
    ytb = nc.dram_tensor("ytb", [DM, L], bf16, kind="ExternalInput")
    wq = nc.dram_tensor("wq", [128, 8 * DPC], bf16, kind="ExternalInput")
    wk = nc.dram_tensor("wk", [128, 8 * DPC], bf16, kind="ExternalInput")
    wv = nc.dram_tensor("wv", [128, 8 * VW], bf16, kind="ExternalInput")
    wo = nc.dram_tensor("wo", [128, 8 * DM], bf16, kind="ExternalInput")
    bqd = nc.dram_tensor("bq", [128, 2], f32, kind="ExternalInput")
    bkd = nc.dram_tensor("bk", [128, 2], f32, kind="ExternalInput")
    bv1 = nc.dram_tensor("bv1", [1, VW], bf16, kind="ExternalInput")
    bod = nc.dram_tensor("bo", [128, 8], f32, kind="ExternalInput")
    mpat = nc.dram_tensor("mpat", [KT, n_pat * SW], bf16, kind="ExternalInput")
    out_t = nc.dram_tensor("out_t", [DM, CHUNK], f32, kind="ExternalOutput")

    # per score-window (qw): which kt are included, and the union span of
    # valid q columns in window coordinates
    def window_kts(qw):
        out = []
        for kt in range(NKT):
            lo, hi = None, None
            for s in range(WW // SW):
                qs = qw * (WW // SW) + s
                if cls[qs][kt]:
                    a, b = span[qs][kt]
                    a += s * SW
                    b += s * SW
                    lo = a if lo is None else min(lo, a)
                    hi = b if hi is None else max(hi, b)
            if lo is not None:
                out.append((kt, lo, hi))
        return out

    with tile.TileContext(nc) as tc:
        with (
            tc.tile_pool(name="const", bufs=1) as cst,
            tc.tile_pool(name="xy", bufs=24) as xy,
            tc.tile_pool(name="big", bufs=1) as big,
            tc.tile_pool(name="exp", bufs=14) as expp,
            tc.tile_pool(name="sm", bufs=4) as sm,
            tc.tile_pool(name="osb", bufs=3) as osb,
            tc.tile_pool(name="rhsp", bufs=8) as rhsp,
            tc.tile_pool(name="sp", bufs=2, space="PSUM") as sp,
            tc.tile_pool(name="avp", bufs=4, space="PSUM") as avp,
            tc.tile_pool(name="dram", bufs=1, space="DRAM") as dram,
            tc.tile_pool(name="dscr", bufs=4, space="DRAM") as dscrp,
        ):
            # ---- wq first: the very first projection needs it ----
            wq_sb = cst.tile([128, 8 * DPC], bf16)
            nc.sync.dma_start(wq_sb[:], wq[:])

            wo_sb = cst.tile([128, 8 * DM], bf16)
            bo_sb = cst.tile([128, 8], f32)

            # start-of-kernel barrier: absorbs per-core launch skew while the
            # big input DMAs stream, so the real AllToAll later isn't skewed
            bar_in = dram.tile([1, 8], f32)
            bar_out = dram.tile([1, 8], f32)
            barrier_sb = cst.tile([1, 8], f32, tag="barrier_sb")
            nc.vector.memset(barrier_sb[:], 0.0)
            nc.sync.dma_start(bar_in[:], barrier_sb[:])
            nc.gpsimd.collective_compute(
                "AllReduce", mybir.AluOpType.add,
                replica_groups=[list(range(NC))],
                ins=[bar_in.opt()], outs=[bar_out.opt()])

            # head-pair p = heads {2p, 2p+1} of this core's slice; each pair
            # shares one 128-row tile (rows 0-63 / 64-127)
            qT = [big.tile([128, L], bf16, name=f"qT{p}") for p in range(2)]
            kT = [big.tile([128, L], bf16, name=f"kT{p}") for p in range(2)]
            vaug = big.tile([128, NKT * VW], bf16)
            headT = [big.tile([128, L], bf16, name=f"hT{p}") for p in range(2)]

            # ---- projections (qc-outer so x and y stream interleaved and
            # all 16 HW DMA queues fill from t=0) ----
            for qc in range(L // 1024):
                xtiles, ytiles = [], []
                for dt in range(8):
                    t = xy.tile([128, 1024], bf16, tag="xy", name=f"x{qc}_{dt}")
                    nc.sync.dma_start(t[:], xtb[dt * 128:(dt + 1) * 128,
                                              qc * 1024:(qc + 1) * 1024])
                    xtiles.append(t)
                for dt in range(8):
                    t = xy.tile([128, 1024], bf16, tag="xy", name=f"y{qc}_{dt}")
                    nc.sync.dma_start(t[:], ytb[dt * 128:(dt + 1) * 128,
                                              qc * 1024:(qc + 1) * 1024])
                    ytiles.append(t)
                if qc == 0:
                    # remaining small constants, queued behind the first input
                    # wave (each a single contiguous DMA)
                    wk_sb = cst.tile([128, 8 * DPC], bf16)
                    wv_sb = cst.tile([128, 8 * VW], bf16)
                    nc.sync.dma_start(wk_sb[:], wk[:])
                    nc.sync.dma_start(wv_sb[:], wv[:])
                    bq_sb = cst.tile([128, 2], f32)
                    bk_sb = cst.tile([128, 2], f32)
                    nc.sync.dma_start(bq_sb[:], bqd[:])
                    nc.sync.dma_start(bk_sb[:], bkd[:])
                    bv1_sb = cst.tile([1, VW], bf16)
                    nc.sync.dma_start(bv1_sb[:], bv1[:])
                    mpat_sb = cst.tile([KT, n_pat * SW], bf16)
                    nc.sync.dma_start(mpat_sb[:], mpat[:])
                    ones_row = cst.tile([1, 128], bf16)
                    nc.vector.memset(ones_row[:], 1.0)
                for s in range(2):
                    col = qc * 1024 + s * SW
                    for src, wsb, bias, dst in ((xtiles, wq_sb, bq_sb, qT),
                                                (ytiles, wk_sb, bk_sb, kT)):
                        for p in range(2):
                            ps = sp.tile([128, WW], f32, tag="sp")
                            for dt in range(8):
                                nc.tensor.matmul(
                                    ps[:128, :SW],
                                    wsb[:, dt * DPC + p * 128:dt * DPC + (p + 1) * 128],
                                    src[dt][:, s * SW:(s + 1) * SW],
                                    start=(dt == 0), stop=(dt == 7))
                            # bias-add on VectorE: keeps ScalarE free for the
                            # exp stream that overlaps the projections
                            nc.vector.tensor_scalar_add(
                                dst[p][:, col:col + SW], ps[:128, :SW],
                                bias[:, p:p + 1])
                for j in range(8):
                    kti = qc * 8 + j
                    psv = avp.tile([128, VW], f32, tag="avp")
                    for dt in range(8):
                        nc.tensor.matmul(psv[:, :VW],
                                         ytiles[dt][:, j * KT:(j + 1) * KT],
                                         wv_sb[:, dt * VW:(dt + 1) * VW],
                                         start=(dt == 0), stop=False)
                    nc.tensor.matmul(psv[:, :VW], ones_row[:],
                                     bv1_sb[:], start=False, stop=True)
                    nc.vector.tensor_copy(vaug[:, kti * VW:kti * VW + VW],
                                          psv[:, :VW])

            # wo/bo issued after all input-stream DMAs (needed only at the end)
            nc.sync.dma_start(wo_sb[:], wo[:])
            nc.sync.dma_start(bo_sb[:], bod[:])

            # ---- attention (this core's batch only) ----
            # Two half A2As, one per score window. Shard j of A2A h = this
            # core's heads for q in [1024h + 128j, 1024h + 128j + 128) of its
            # batch. Core j then holds full depth of q-range 1024h + 128j for
            # BOTH batches (shards 0-3 carry batch 0, 4-7 carry batch 1), so
            # A2A#1 and the first output-projection half overlap window 1.
            QQ = 128
            a2aA_in = dram.tile([NC, DPC, QQ], bf16)
            a2aA_out = dram.tile([NC, DPC, QQ], bf16)
            a2aB_in = dram.tile([NC, DPC, QQ], bf16)
            a2aB_out = dram.tile([NC, DPC, QQ], bf16)

            def emit_oproj(half, aout):
                rhs = []
                for r in range(8):
                    jj, sub = r // 2, r % 2
                    t = rhsp.tile([128, 2 * QQ], bf16, tag="a2a",
                                  name=f"a2a{half}_{r}")
                    nc.sync.dma_start(t[:, 0:QQ],
                                      aout[jj][sub * 128:(sub + 1) * 128, :])
                    nc.sync.dma_start(t[:, QQ:2 * QQ],
                                      aout[jj + 4][sub * 128:(sub + 1) * 128, :])
                    rhs.append(t)
                for mt in range(8):
                    ps = sp.tile([128, WW], f32, tag="sp", name=f"po{half}_{mt}")
                    for r in range(8):
                        nc.tensor.matmul(
                            ps[:, :2 * QQ],
                            wo_sb[:, r * DM + mt * 128:r * DM + (mt + 1) * 128],
                            rhs[r][:],
                            start=(r == 0), stop=(r == 7))
                    ob = osb.tile([128, 2 * QQ], f32, tag="osb")
                    nc.vector.tensor_scalar_add(ob[:], ps[:, :2 * QQ],
                                                bo_sb[:, mt:mt + 1])
                    nc.sync.dma_start(
                        out_t[mt * 128:(mt + 1) * 128,
                              half * 2 * QQ:(half + 1) * 2 * QQ], ob[:])
            for qw in range(NWW):
                    wkts = window_kts(qw)
                    wcol = qw * WW
                    # last kt per sub-window (closes that AV chain)
                    lastkt = {}
                    for s in range(WW // SW):
                        qs = qw * (WW // SW) + s
                        inc = [kt for kt, _, _ in wkts if cls[qs][kt]]
                        if inc:
                            lastkt[s] = inc[-1]
                    # one pass per head-pair: scores -> exp -> AV per kt, so
                    # exp tiles die immediately and AV chains (2 qs halves x
                    # 2 heads = 4 PSUM banks) stay open across the kt loop
                    for p in range(2):
                        av_t = {}
                        for kt, lo, hi in wkts:
                            pss = []
                            for hp in range(2):
                                hs = hp * DK
                                ps = sp.tile([128, WW], f32, tag="sp", name=f"ps{hp}")
                                for a, b in ((lo, min(hi, SW)), (max(lo, SW), hi)):
                                    if a >= b:
                                        continue
                                    nc.tensor.matmul(
                                        ps[:KT, a:b],
                                        kT[p][hs:hs + DK, kt * KT:(kt + 1) * KT],
                                        qT[p][hs:hs + DK, wcol + a:wcol + b],
                                        start=True, stop=True)
                                pss.append(ps)
                            ets = []
                            for hp in range(2):
                                et = expp.tile([KT, WW], bf16, tag="exp")
                                nc.scalar.activation(et[:, lo:hi], pss[hp][:KT, lo:hi],
                                                     mybir.ActivationFunctionType.Exp)
                                ets.append(et)
                            for s in range(WW // SW):
                                qs = qw * (WW // SW) + s
                                if not cls[qs][kt]:
                                    continue
                                a, b = span[qs][kt]
                                for hp in range(2):
                                    h = 2 * p + hp  # head within this core
                                    et = ets[hp]
                                    if cls[qs][kt] == 2:
                                        pi = pat_idx[qs][kt]
                                        nc.vector.tensor_tensor(
                                            et[:, s * SW + a:s * SW + b],
                                            et[:, s * SW + a:s * SW + b],
                                            mpat_sb[:, pi * SW + a:pi * SW + b],
                                            mybir.AluOpType.mult)
                                    first = (s, hp) not in av_t
                                    if first:
                                        av_t[s, hp] = avp.tile(
                                            [65, SW], f32, tag="avp",
                                            name=f"av{s}{hp}")
                                    av = av_t[s, hp]
                                    done = kt == lastkt[s]
                                    nc.tensor.matmul(
                                        av[:, a:b],
                                        vaug[:, kt * VW + h * 65:kt * VW + (h + 1) * 65],
                                        et[:, s * SW + a:s * SW + b],
                                        start=first, stop=done)
                                    if not done:
                                        continue
                                    # chain closed: stage to SBUF immediately
                                    # (frees the PSUM bank), then normalize:
                                    # reshape den row across lanes, exact
                                    # reciprocal, broadcast via stride-0 DMA.
                                    stg = sm.tile([65, SW], f32, tag="stg")
                                    nc.vector.tensor_copy(stg[:], av[:, :])
                                    d128 = sm.tile([128, SW // 128], f32, tag="d128")
                                    nc.sync.dma_start(d128[:], stg[64:65, :])
                                    r128 = sm.tile([128, SW // 128], f32, tag="r128")
                                    nc.vector.reciprocal(r128[:], d128[:])
                                    dsc = dscrp.tile([1, SW], f32, tag="dscr")
                                    nc.sync.dma_start(dsc[:], r128[:])
                                    bcs = sm.tile([DK, SW], f32, tag="bcs")
                                    nc.sync.dma_start(bcs[:], dsc[:].to_broadcast([DK, SW]))
                                    ccol = qs * SW
                                    nc.vector.tensor_tensor(
                                        headT[p][hp * DK:(hp + 1) * DK, ccol:ccol + SW],
                                        stg[:DK, :], bcs[:],
                                        mybir.AluOpType.mult)
                    # all 4 heads done for this window: ship its 8 shards and
                    # fire this window's half A2A; the first half's output
                    # projection overlaps window 1's attention
                    ain = a2aA_in if qw == 0 else a2aB_in
                    for j in range(8):
                        for p in range(2):
                            nc.sync.dma_start(
                                ain[j][p * 128:(p + 1) * 128, :],
                                headT[p][:, qw * WW + j * QQ:qw * WW + (j + 1) * QQ])
                    if qw == 0:
                        nc.gpsimd.collective_compute(
                            "AllToAll", mybir.AluOpType.bypass,
                            replica_groups=[list(range(NC))],
                            ins=[a2aA_in.opt()], outs=[a2aA_out.opt()])
                    else:
                        nc.gpsimd.collective_compute(
                            "AllToAll", mybir.AluOpType.bypass,
                            replica_groups=[list(range(NC))],
                            ins=[a2aB_in.opt()], outs=[a2aB_out.opt()])

            # first output-projection half: emitted after the attention loop
            # so its PSUM-slot requests rank BELOW window 1's scores (slot
            # grants follow program order), but its inputs landed with A2A#1
            # so it fills the window-1 tail and the A2A#2 wait
            emit_oproj(0, a2aA_out)

            # keep the PE clock warm during the final collective wait (gated
            # on the LAST-written headT columns = end of attention)
            for i in range(14):
                wps = sp.tile([128, WW], f32, tag="sp", name=f"warm{i}")
                nc.tensor.matmul(wps[:, :SW], headT[1][:, L - 128:L],
                                 headT[1][:, L - SW:L],
                                 start=True, stop=True)

            emit_oproj(1, a2aB_out)

    nc.compile()
    return nc


def kernel(x, y, mask, Wq, bq, Wk, bk, Wv, bv, Wo, bo, _trace=False):
    x = np.asarray(x, np.float32)
    y = np.asarray(y, np.float32)
    cls, span, pat_idx, pats = _classify_blocks(mask)

    key = (x.shape,
           tuple(tuple(c) for c in cls),
           tuple(tuple(s) for s in span),
           tuple(tuple(p) for p in pat_idx),
           pats.tobytes())
    if key not in _CACHE:
        _CACHE[key] = _build(cls, span, pat_idx, pats.shape[0])
    nc = _CACHE[key]

    fac = np.float32(1.0 / np.sqrt(DK))
    xtb = [np.ascontiguousarray(x[n].T).astype(BF16) for n in range(NB)]
    ytb = [np.ascontiguousarray(y[n].T).astype(BF16) for n in range(NB)]
    Wq32 = np.asarray(Wq, np.float32) * fac
    bq32 = np.asarray(bq, np.float32) * fac

    def fold(w):
        # [DM, C] -> [128, 8*C]: dm-block dt lands at columns [dt*C, (dt+1)*C)
        c = w.shape[1]
        return np.ascontiguousarray(
            w.reshape(8, 128, c).transpose(1, 0, 2).reshape(128, 8 * c))

    mpat_f = np.ascontiguousarray(
        pats.transpose(1, 0, 2).reshape(KT, -1))  # [128, n_pat*SW]

    in_maps = []
    for c in range(NC):
        b, g = c // GPB, c % GPB
        d0 = g * DPC  # this core's depth slice (heads 4g..4g+3)
        wv_aug = np.zeros((DM, VW), np.float32)
        bv1 = np.zeros((1, VW), np.float32)
        for hp in range(HPC):
            h = HPC * g + hp
            wv_aug[:, hp * 65:hp * 65 + DK] = np.asarray(Wv, np.float32)[:, h * DK:(h + 1) * DK]
            bv1[0, hp * 65:hp * 65 + DK] = np.asarray(bv, np.float32)[h * DK:(h + 1) * DK]
            bv1[0, hp * 65 + DK] = 1.0
        in_maps.append({
            "xtb": xtb[b], "ytb": ytb[b],
            "wq": fold(Wq32[:, d0:d0 + DPC].astype(BF16)),
            "wk": fold(np.asarray(Wk, np.float32)[:, d0:d0 + DPC].astype(BF16)),
            "wv": fold(wv_aug.astype(BF16)),
            "wo": fold(np.asarray(Wo, np.float32).astype(BF16)),
            "bq": np.ascontiguousarray(bq32[d0:d0 + DPC].reshape(2, 128).T),
            "bk": np.ascontiguousarray(
                np.asarray(bk, np.float32)[d0:d0 + DPC].reshape(2, 128).T),
            "bv1": bv1.astype(BF16),
            "bo": np.ascontiguousarray(
                np.asarray(bo, np.float32).reshape(8, 128).T),
            "mpat": mpat_f,
        })

    res = run_bass_kernel_spmd(nc, in_maps, core_ids=list(range(NC)), trace=_trace)
    out = np.empty((NB, L, DM), np.float32)
    QQ = 128  # q rows per core per window half
    for c in range(NC):
        ot = res.results[c]["out_t"]
        for half in range(2):
            q0 = half * (L // 2) + QQ * c
            out[0, q0:q0 + QQ, :] = ot[:, half * 2 * QQ:half * 2 * QQ + QQ].T
            out[1, q0:q0 + QQ, :] = ot[:, half * 2 * QQ + QQ:half * 2 * QQ + 2 * QQ].T
    if _trace:
        kernel.last_results = res
    return out


# revision 40
# speedup vs baseline: 1.2864x; 1.0449x over previous
"""Multi-head attention (N=2, L=2048, 16 heads x 64) on 8 TRN2 NeuronCores.

Sharding: batch x head-group. Core c handles batch c//4 and heads
4*(c%4)..4*(c%4)+3, so each core streams only its own batch's x/y (8.4 MB
instead of 16.8 MB) and computes all four projections for its head slice.
One AllToAll per 4-core batch group (replica_groups [[0-3],[4-7]]) reshards
from head-split to sequence-split for the output projection.

All matmuls bf16 with fp32 accumulation; softmax in fp32 (exp on ScalarE,
denominator via a ones-column appended to V). Scores are computed
transposed ([k, q]); attention weights feed the AV matmul as the moving
operand. The AV output [dv+1, q] carries the denominator in row 64;
normalization is 1/den broadcast via DMA and applied on VectorE. The
output projection produces the final output transposed; the host
transposes back.
"""
import sys

sys.path.insert(0, "/opt/trn_rl_repo")

import numpy as np
import ml_dtypes

import concourse.bass as bass
import concourse.bacc as bacc
import concourse.mybir as mybir
import concourse.tile as tile
from concourse.bass_utils import run_bass_kernel_spmd

BF16 = ml_dtypes.bfloat16

DM = 1024      # dmodel
DK = 64        # head dim
H = 16         # heads
NB = 2         # batch
L = 2048       # seq len
NC = 8         # cores
GPB = NC // NB  # head-groups (cores) per batch = 4
HPC = H // GPB  # heads per core = 4
DPC = HPC * DK  # depth per core = 256
VW = HPC * 65   # v_aug width (4 heads x (64+1 ones col)) = 260

SW = 512       # sub-window (AV, masks, output chunk)
WW = 1024      # score/exp window
KT = 128       # k tile
NSW = L // SW   # 4 sub-windows
NWW = L // WW   # 2 score windows
NKT = L // KT   # 16 k tiles
CHUNK = L // GPB  # 512 q rows per core output

_CACHE = {}


def _classify_blocks(mask):
    """Per 512-granular (qs, kt): 0=skip, 1=full, 2=partial (+ q-span, pattern)."""
    mask = np.asarray(mask, dtype=bool)
    cls = [[0] * NKT for _ in range(NSW)]
    span = [[None] * NKT for _ in range(NSW)]
    pat_ids = {}
    pats = []
    pat_idx = [[-1] * NKT for _ in range(NSW)]
    for qs in range(NSW):
        for kt in range(NKT):
            sub = mask[qs * SW:(qs + 1) * SW, kt * KT:(kt + 1) * KT]
            rows = np.nonzero(sub.any(axis=1))[0]
            if rows.size == 0:
                cls[qs][kt] = 0
            elif sub.all():
                cls[qs][kt] = 1
                span[qs][kt] = (0, SW)
            else:
                cls[qs][kt] = 2
                span[qs][kt] = (int(rows[0]), int(rows[-1]) + 1)
                pat = np.ascontiguousarray(sub.T).astype(BF16)  # [128 k, SW q]
                key = pat.tobytes()
                if key not in pat_ids:
                    pat_ids[key] = len(pats)
                    pats.append(pat)
                pat_idx[qs][kt] = pat_ids[key]
    # general-mask safety: the first included kt of each sub-window must cover
    # the full 512 columns (its start=True matmul clears PSUM has_written)
    for qs in range(NSW):
        for kt in range(NKT):
            if cls[qs][kt]:
                span[qs][kt] = (0, SW)
                break
    if not pats:
        pats.append(np.ones((KT, SW), dtype=BF16))
    return cls, span, pat_idx, np.stack(pats)


def _build(cls, span, pat_idx, n_pat):
    nc = bacc.Bacc("TRN2", target_bir_lowering=False, debug=False,
                   enable_asserts=False, num_devices=NC)
    f32, bf16 = mybir.dt.float32, mybir.dt.bfloat16

    # weights arrive host-pretransposed as [128, 8*cols] so each loads in a
    # single contiguous DMA
    xtb = nc.dram_tensor("xtb", [DM, L], bf16, kind="ExternalInput")
    ytb = nc.dram_tensor("ytb", [DM, L], bf16, kind="ExternalInput")
    wq = nc.dram_tensor("wq", [128, 8 * DPC], bf16, kind="ExternalInput")
    wk = nc.dram_tensor("wk", [128, 8 * DPC], bf16, kind="ExternalInput")
    wv = nc.dram_tensor("wv", [128, 8 * VW], bf16, kind="ExternalInput")
    wo = nc.dram_tensor("wo", [128, 8 * DM], bf16, kind="ExternalInput")
    bqd = nc.dram_tensor("bq", [128, 2], f32, kind="ExternalInput")
    bkd = nc.dram_tensor("bk", [128, 2], f32, kind="ExternalInput")
    bv1 = nc.dram_tensor("bv1", [1, VW], bf16, kind="ExternalInput")
    bod = nc.dram_tensor("bo", [128, 8], f32, kind="ExternalInput")
    mpat = nc.dram_tensor("mpat", [KT, n_pat * SW], bf16, kind="ExternalInput")
    out_t = nc.dram_tensor("out_t", [DM, CHUNK], f32, kind="ExternalOutput")

    # per score-window (qw): which kt are included, and the union span of
    # valid q columns in window coordinates
    def window_kts(qw):
        out = []
        for kt in range(NKT):
            lo, hi = None, None
            for s in range(WW // SW):
                qs = qw * (WW // SW) + s
                if cls[qs][kt]:
                    a, b = span[qs][kt]
                    a += s * SW
                    b += s * SW
                    lo = a if lo is None else min(lo, a)
                    hi = b if hi is None else max(hi, b)
            if lo is not None:
                out.append((kt, lo, hi))
        return out

    with tile.TileContext(nc) as tc:
        with (
            tc.tile_pool(name="const", bufs=1) as cst,
            tc.tile_pool(name="xy", bufs=24) as xy,
            tc.tile_pool(name="big", bufs=1) as big,
            tc.tile_pool(name="exp", bufs=14) as expp,
            tc.tile_pool(name="sm", bufs=4) as sm,
            tc.tile_pool(name="osb", bufs=3) as osb,
            tc.tile_pool(name="rhsp", bufs=8) as rhsp,
            tc.tile_pool(name="sp", bufs=2, space="PSUM") as sp,
            tc.tile_pool(name="avp", bufs=4, space="PSUM") as avp,
            tc.tile_pool(name="dram", bufs=1, space="DRAM") as dram,
            tc.tile_pool(name="dscr", bufs=4, space="DRAM") as dscrp,
        ):
            # ---- wq first: the very first projection needs it. Split across
            # DMA queues (a single queue only sustains ~18 GB/s) ----
            wq_sb = cst.tile([128, 8 * DPC], bf16)
            for i in range(4):
                w = 2 * DPC
                nc.sync.dma_start(wq_sb[:, i * w:(i + 1) * w], wq[:, i * w:(i + 1) * w])

            wo_sb = cst.tile([128, 8 * DM], bf16)
            bo_sb = cst.tile([128, 8], f32)

            # start-of-kernel barrier: absorbs per-core launch skew while the
            # big input DMAs stream, so the real AllToAll later isn't skewed
            bar_in = dram.tile([1, 8], f32)
            bar_out = dram.tile([1, 8], f32)
            barrier_sb = cst.tile([1, 8], f32, tag="barrier_sb")
            nc.vector.memset(barrier_sb[:], 0.0)
            nc.sync.dma_start(bar_in[:], barrier_sb[:])
            nc.gpsimd.collective_compute(
                "AllReduce", mybir.AluOpType.add,
                replica_groups=[list(range(NC))],
                ins=[bar_in.opt()], outs=[bar_out.opt()])

            # head-pair p = heads {2p, 2p+1} of this core's slice; each pair
            # shares one 128-row tile (rows 0-63 / 64-127)
            qT = [big.tile([128, L], bf16, name=f"qT{p}") for p in range(2)]
            kT = [big.tile([128, L], bf16, name=f"kT{p}") for p in range(2)]
            vaug = big.tile([128, NKT * VW], bf16)
            headT = [big.tile([128, L], bf16, name=f"hT{p}") for p in range(2)]

            # ---- projections (qc-outer so x and y stream interleaved and
            # all 16 HW DMA queues fill from t=0) ----
            for qc in range(L // 1024):
                xtiles, ytiles = [], []
                for dt in range(8):
                    t = xy.tile([128, 1024], bf16, tag="xy", name=f"x{qc}_{dt}")
                    nc.sync.dma_start(t[:], xtb[dt * 128:(dt + 1) * 128,
                                              qc * 1024:(qc + 1) * 1024])
                    xtiles.append(t)
                for dt in range(8):
                    t = xy.tile([128, 1024], bf16, tag="xy", name=f"y{qc}_{dt}")
                    nc.sync.dma_start(t[:], ytb[dt * 128:(dt + 1) * 128,
                                              qc * 1024:(qc + 1) * 1024])
                    ytiles.append(t)
                if qc == 0:
                    # remaining small constants, queued behind the first input
                    # wave (each a single contiguous DMA)
                    wk_sb = cst.tile([128, 8 * DPC], bf16)
                    wv_sb = cst.tile([128, 8 * VW], bf16)
                    for i in range(4):
                        w = 2 * DPC
                        nc.sync.dma_start(wk_sb[:, i * w:(i + 1) * w],
                                          wk[:, i * w:(i + 1) * w])
                        w = 2 * VW
                        nc.sync.dma_start(wv_sb[:, i * w:(i + 1) * w],
                                          wv[:, i * w:(i + 1) * w])
                    bq_sb = cst.tile([128, 2], f32)
                    bk_sb = cst.tile([128, 2], f32)
                    nc.sync.dma_start(bq_sb[:], bqd[:])
                    nc.sync.dma_start(bk_sb[:], bkd[:])
                    bv1_sb = cst.tile([1, VW], bf16)
                    nc.sync.dma_start(bv1_sb[:], bv1[:])
                    mpat_sb = cst.tile([KT, n_pat * SW], bf16)
                    for i in range(n_pat):
                        nc.sync.dma_start(mpat_sb[:, i * SW:(i + 1) * SW],
                                          mpat[:, i * SW:(i + 1) * SW])
                    ones_row = cst.tile([1, 128], bf16)
                    nc.vector.memset(ones_row[:], 1.0)
                for s in range(2):
                    col = qc * 1024 + s * SW
                    for src, wsb, bias, dst in ((xtiles, wq_sb, bq_sb, qT),
                                                (ytiles, wk_sb, bk_sb, kT)):
                        for p in range(2):
                            ps = sp.tile([128, WW], f32, tag="sp")
                            for dt in range(8):
                                nc.tensor.matmul(
                                    ps[:128, :SW],
                                    wsb[:, dt * DPC + p * 128:dt * DPC + (p + 1) * 128],
                                    src[dt][:, s * SW:(s + 1) * SW],
                                    start=(dt == 0), stop=(dt == 7))
                            # bias-add on VectorE: keeps ScalarE free for the
                            # exp stream that overlaps the projections
                            nc.vector.tensor_scalar_add(
                                dst[p][:, col:col + SW], ps[:128, :SW],
                                bias[:, p:p + 1])
                for j in range(8):
                    kti = qc * 8 + j
                    psv = avp.tile([128, VW], f32, tag="avp")
                    for dt in range(8):
                        nc.tensor.matmul(psv[:, :VW],
                                         ytiles[dt][:, j * KT:(j + 1) * KT],
                                         wv_sb[:, dt * VW:(dt + 1) * VW],
                                         start=(dt == 0), stop=False)
                    nc.tensor.matmul(psv[:, :VW], ones_row[:],
                                     bv1_sb[:], start=False, stop=True)
                    nc.vector.tensor_copy(vaug[:, kti * VW:kti * VW + VW],
                                          psv[:, :VW])

            # wo/bo issued after all input-stream DMAs (needed only at the end)
            for i in range(8):
                nc.sync.dma_start(wo_sb[:, i * DM:(i + 1) * DM],
                                  wo[:, i * DM:(i + 1) * DM])
            nc.sync.dma_start(bo_sb[:], bod[:])

            # ---- attention (this core's batch only) ----
            # Two half A2As, one per score window. Shard j of A2A h = this
            # core's heads for q in [1024h + 128j, 1024h + 128j + 128) of its
            # batch. Core j then holds full depth of q-range 1024h + 128j for
            # BOTH batches (shards 0-3 carry batch 0, 4-7 carry batch 1), so
            # A2A#1 and the first output-projection half overlap window 1.
            QQ = 128
            a2aA_in = dram.tile([NC, DPC, QQ], bf16)
            a2aA_out = dram.tile([NC, DPC, QQ], bf16)
            a2aB_in = dram.tile([NC, DPC, QQ], bf16)
            a2aB_out = dram.tile([NC, DPC, QQ], bf16)

            def emit_oproj(half, aout):
                rhs = []
                for r in range(8):
                    jj, sub = r // 2, r % 2
                    t = rhsp.tile([128, 2 * QQ], bf16, tag="a2a",
                                  name=f"a2a{half}_{r}")
                    nc.sync.dma_start(t[:, 0:QQ],
                                      aout[jj][sub * 128:(sub + 1) * 128, :])
                    nc.sync.dma_start(t[:, QQ:2 * QQ],
                                      aout[jj + 4][sub * 128:(sub + 1) * 128, :])
                    rhs.append(t)
                for mt in range(8):
                    ps = sp.tile([128, WW], f32, tag="sp", name=f"po{half}_{mt}")
                    for r in range(8):
                        nc.tensor.matmul(
                            ps[:, :2 * QQ],
                            wo_sb[:, r * DM + mt * 128:r * DM + (mt + 1) * 128],
                            rhs[r][:],
                            start=(r == 0), stop=(r == 7))
                    ob = osb.tile([128, 2 * QQ], f32, tag="osb")
                    nc.vector.tensor_scalar_add(ob[:], ps[:, :2 * QQ],
                                                bo_sb[:, mt:mt + 1])
                    nc.sync.dma_start(
                        out_t[mt * 128:(mt + 1) * 128,
                              half * 2 * QQ:(half + 1) * 2 * QQ], ob[:])
            for qw in range(NWW):
                    wkts = window_kts(qw)
                    wcol = qw * WW
                    # last kt per sub-window (closes that AV chain)
                    lastkt = {}
                    for s in range(WW // SW):
                        qs = qw * (WW // SW) + s
                        inc = [kt for kt, _, _ in wkts if cls[qs][kt]]
                        if inc:
                            lastkt[s] = inc[-1]
                    # one pass per head-pair: scores -> exp -> AV per kt, so
                    # exp tiles die immediately and AV chains (2 qs halves x
                    # 2 heads = 4 PSUM banks) stay open across the kt loop
                    for p in range(2):
                        av_t = {}
                        for kt, lo, hi in wkts:
                            pss = []
                            for hp in range(2):
                                hs = hp * DK
                                ps = sp.tile([128, WW], f32, tag="sp", name=f"ps{hp}")
                                for a, b in ((lo, min(hi, SW)), (max(lo, SW), hi)):
                                    if a >= b:
                                        continue
                                    nc.tensor.matmul(
                                        ps[:KT, a:b],
                                        kT[p][hs:hs + DK, kt * KT:(kt + 1) * KT],
                                        qT[p][hs:hs + DK, wcol + a:wcol + b],
                                        start=True, stop=True)
                                pss.append(ps)
                            ets = []
                            for hp in range(2):
                                et = expp.tile([KT, WW], bf16, tag="exp")
                                nc.scalar.activation(et[:, lo:hi], pss[hp][:KT, lo:hi],
                                                     mybir.ActivationFunctionType.Exp)
                                ets.append(et)
                            for s in range(WW // SW):
                                qs = qw * (WW // SW) + s
                                if not cls[qs][kt]:
                                    continue
                                a, b = span[qs][kt]
                                for hp in range(2):
                                    h = 2 * p + hp  # head within this core
                                    et = ets[hp]
                                    if cls[qs][kt] == 2:
                                        pi = pat_idx[qs][kt]
                                        nc.vector.tensor_tensor(
                                            et[:, s * SW + a:s * SW + b],
                                            et[:, s * SW + a:s * SW + b],
                                            mpat_sb[:, pi * SW + a:pi * SW + b],
                                            mybir.AluOpType.mult)
                                    first = (s, hp) not in av_t
                                    if first:
                                        av_t[s, hp] = avp.tile(
                                            [65, SW], f32, tag="avp",
                                            name=f"av{s}{hp}")
                                    av = av_t[s, hp]
                                    done = kt == lastkt[s]
                                    nc.tensor.matmul(
                                        av[:, a:b],
                                        vaug[:, kt * VW + h * 65:kt * VW + (h + 1) * 65],
                                        et[:, s * SW + a:s * SW + b],
                                        start=first, stop=done)
                                    if not done:
                                        continue
                                    # chain closed: stage to SBUF immediately
                                    # (frees the PSUM bank), then normalize:
                                    # reshape den row across lanes, exact
                                    # reciprocal, broadcast via stride-0 DMA.
                                    stg = sm.tile([65, SW], f32, tag="stg")
                                    nc.vector.tensor_copy(stg[:], av[:, :])
                                    d128 = sm.tile([128, SW // 128], f32, tag="d128")
                                    nc.sync.dma_start(d128[:], stg[64:65, :])
                                    r128 = sm.tile([128, SW // 128], f32, tag="r128")
                                    nc.vector.reciprocal(r128[:], d128[:])
                                    dsc = dscrp.tile([1, SW], f32, tag="dscr")
                                    nc.sync.dma_start(dsc[:], r128[:])
                                    bcs = sm.tile([DK, SW], f32, tag="bcs")
                                    nc.sync.dma_start(bcs[:], dsc[:].to_broadcast([DK, SW]))
                                    ccol = qs * SW
                                    nc.vector.tensor_tensor(
                                        headT[p][hp * DK:(hp + 1) * DK, ccol:ccol + SW],
                                        stg[:DK, :], bcs[:],
                                        mybir.AluOpType.mult)
                    # all 4 heads done for this window: ship its 8 shards and
                    # fire this window's half A2A; the first half's output
                    # projection overlaps window 1's attention
                    ain = a2aA_in if qw == 0 else a2aB_in
                    for j in range(8):
                        for p in range(2):
                            nc.sync.dma_start(
                                ain[j][p * 128:(p + 1) * 128, :],
                                headT[p][:, qw * WW + j * QQ:qw * WW + (j + 1) * QQ])
                    if qw == 0:
                        nc.gpsimd.collective_compute(
                            "AllToAll", mybir.AluOpType.bypass,
                            replica_groups=[list(range(NC))],
                            ins=[a2aA_in.opt()], outs=[a2aA_out.opt()])
                    else:
                        nc.gpsimd.collective_compute(
                            "AllToAll", mybir.AluOpType.bypass,
                            replica_groups=[list(range(NC))],
                            ins=[a2aB_in.opt()], outs=[a2aB_out.opt()])

            # first output-projection half: emitted after the attention loop
            # so its PSUM-slot requests rank BELOW window 1's scores (slot
            # grants follow program order), but its inputs landed with A2A#1
            # so it fills the window-1 tail and the A2A#2 wait
            emit_oproj(0, a2aA_out)

            # keep the PE clock warm during the final collective wait (gated
            # on the LAST-written headT columns = end of attention)
            for i in range(28):
                wps = sp.tile([128, WW], f32, tag="sp", name=f"warm{i}")
                nc.tensor.matmul(wps[:, :SW], headT[1][:, L - 128:L],
                                 headT[1][:, L - SW:L],
                                 start=True, stop=True)

            emit_oproj(1, a2aB_out)

    nc.compile()
    return nc


def kernel(x, y, mask, Wq, bq, Wk, bk, Wv, bv, Wo, bo, _trace=False):
    x = np.asarray(x, np.float32)
    y = np.asarray(y, np.float32)
    cls, span, pat_idx, pats = _classify_blocks(mask)

    key = (x.shape,
           tuple(tuple(c) for c in cls),
           tuple(tuple(s) for s in span),
           tuple(tuple(p) for p in pat_idx),
           pats.tobytes())
    if key not in _CACHE:
        _CACHE[key] = _build(cls, span, pat_idx, pats.shape[0])
    nc = _CACHE[key]

    fac = np.float32(1.0 / np.sqrt(DK))
    xtb = [np.ascontiguousarray(x[n].T).astype(BF16) for n in range(NB)]
    ytb = [np.ascontiguousarray(y[n].T).astype(BF16) for n in range(NB)]
    Wq32 = np.asarray(Wq, np.float32) * fac
    bq32 = np.asarray(bq, np.float32) * fac

    def fold(w):
        # [DM, C] -> [128, 8*C]: dm-block dt lands at columns [dt*C, (dt+1)*C)
        c = w.shape[1]
        return np.ascontiguousarray(
            w.reshape(8, 128, c).transpose(1, 0, 2).reshape(128, 8 * c))

    mpat_f = np.ascontiguousarray(
        pats.transpose(1, 0, 2).reshape(KT, -1))  # [128, n_pat*SW]

    in_maps = []
    for c in range(NC):
        b, g = c // GPB, c % GPB
        d0 = g * DPC  # this core's depth slice (heads 4g..4g+3)
        wv_aug = np.zeros((DM, VW), np.float32)
        bv1 = np.zeros((1, VW), np.float32)
        for hp in range(HPC):
            h = HPC * g + hp
            wv_aug[:, hp * 65:hp * 65 + DK] = np.asarray(Wv, np.float32)[:, h * DK:(h + 1) * DK]
            bv1[0, hp * 65:hp * 65 + DK] = np.asarray(bv, np.float32)[h * DK:(h + 1) * DK]
            bv1[0, hp * 65 + DK] = 1.0
        in_maps.append({
            "xtb": xtb[b], "ytb": ytb[b],
            "wq": fold(Wq32[:, d0:d0 + DPC].astype(BF16)),
            "wk": fold(np.asarray(Wk, np.float32)[:, d0:d0 + DPC].astype(BF16)),
            "wv": fold(wv_aug.astype(BF16)),
            "wo": fold(np.asarray(Wo, np.float32).astype(BF16)),
            "bq": np.ascontiguousarray(bq32[d0:d0 + DPC].reshape(2, 128).T),
            "bk": np.ascontiguousarray(
                np.asarray(bk, np.float32)[d0:d0 + DPC].reshape(2, 128).T),
            "bv1": bv1.astype(BF16),
            "bo": np.ascontiguousarray(
                np.asarray(bo, np.float32).reshape(8, 128).T),
            "mpat": mpat_f,
        })

    res = run_bass_kernel_spmd(nc, in_maps, core_ids=list(range(NC)), trace=_trace)
    out = np.empty((NB, L, DM), np.float32)
    QQ = 128  # q rows per core per window half
    for c in range(NC):
        ot = res.results[c]["out_t"]
        for half in range(2):
            q0 = half * (L // 2) + QQ * c
            out[0, q0:q0 + QQ, :] = ot[:, half * 2 * QQ:half * 2 * QQ + QQ].T
            out[1, q0:q0 + QQ, :] = ot[:, half * 2 * QQ + QQ:half * 2 * QQ + 2 * QQ].T
    if _trace:
        kernel.last_results = res
    return out
